# revision 1
# baseline (speedup 1.0000x reference)
"""Distributed Trainium2 (8 NeuronCores) kernel for masked multi-head attention
+ output projection (nn_Attention_60790967107825).

Head-parallel attention, row-parallel projection, one AllToAll between:
  - Each core owns 2 of the 16 heads (all 4 batches) -> 8 (b,h) pairs/core.
  - Host prep: q/k fed pre-transposed per head (so the TensorE contraction
    needs no on-device transposes), everything cast to bf16, and keys
    COMPACTED per batch to the unmasked set (masked keys contribute
    exp(-inf)=0 exactly), padded to a multiple of 128; pad slots carry
    k=0 and a 0 in the ones-column appended to V, so they add 0 to both
    the PV numerator and the softmax denominator. ~2x less exp/matmul work
    at ~50% mask density. The ones-column also yields the denominators as
    row 64 of the PV accumulation for free.
  - Scores are computed transposed (S^T[j,i] = K Q^T) so P^T feeds the PV
    matmul directly as the moving operand with V stationary.
  - Numerators are scaled by m_i/Z_i (DVE fast reciprocal + DMA partition
    broadcast); the masked-query uniform-attention term is rank-1 per batch
    and is re-added after the projection from a V-mean column carried
    through the collective (u[r] * (Vmean_b @ W^T) + b_out).
  - Two AllToAlls (~1.1MB/rank each), one per head-half: the first issues
    halfway through attention and hides completely; PE warm-up matmuls
    bridge the second so the projection runs at full HAM clock.
  - After the exchange every core holds all 1024 channels for its own 1024
    output rows at identical local addresses (SPMD-clean) and runs the
    full projection locally; outputs concatenate on the host.
"""

import os
import sys

import numpy as np

for _p in ("/opt/trn_rl_repo", "/root/.axon_site/_ro/trn_rl_repo"):
    if os.path.isdir(_p) and _p not in sys.path:
        sys.path.insert(0, _p)

import ml_dtypes  # noqa: E402
import concourse.bass as bass  # noqa: E402,F401
import concourse.mybir as mybir  # noqa: E402
import concourse.tile as tile  # noqa: E402
from concourse import bacc  # noqa: E402
from concourse.bass_utils import run_bass_kernel_spmd  # noqa: E402

B, H, N, D = 4, 16, 2048, 64
DIM = H * D
P = 128
NCORES = 8
HPC = H // NCORES          # heads per core
PAIRS = B * HPC            # (b, h_local) pairs per core
SCALE = float(D) ** -0.5
IC = 2                     # query chunks per pair
ICW = N // IC              # 1024
RB = B * N // NCORES       # 1024 output rows per core
RBW = RB + 16              # a2a row width (col RB carries the V-mean)
CT = DIM // P              # 8 contraction tiles in the projection
MBIG = 1.0e30              # Z multiplier for masked queries -> 1/Z == 0

bf16 = mybir.dt.bfloat16
f32 = mybir.dt.float32
npbf = ml_dtypes.bfloat16

_CACHE = {}



def build_graph(nkb=(N,) * B):
    nk = max(nkb)
    nc = bacc.Bacc("TRN2", num_devices=NCORES)

    qT = nc.dram_tensor("qT", [PAIRS, D, N], bf16, kind="ExternalInput")
    kT = nc.dram_tensor("kT", [PAIRS, D, nk], bf16, kind="ExternalInput")
    vv = nc.dram_tensor("v", [PAIRS, nk, D + 1], bf16, kind="ExternalInput")
    minvD = nc.dram_tensor("minv", [PAIRS, N], f32, kind="ExternalInput")
    uD = nc.dram_tensor("uproj", [P, RB // P], f32, kind="ExternalInput")
    wTD = nc.dram_tensor("wT", [DIM, DIM], bf16, kind="ExternalInput")
    boutD = nc.dram_tensor("bout", [1, DIM], f32, kind="ExternalInput")
    vmD = nc.dram_tensor("vmean", [PAIRS, D], bf16, kind="ExternalInput")
    outD = nc.dram_tensor("out", [RB, DIM], f32, kind="ExternalOutput")

    with tile.TileContext(nc, num_cores=NCORES) as tc:
        with tc.tile_pool(name="dram", bufs=1, space="DRAM") as dramp:
            # one buffer pair per head-half: A2A #0 (h_local=0 heads) issues
            # after half the pairs and hides under the remaining compute
            a2a_in = [
                dramp.tile([NCORES, D, RBW], bf16, name=f"a2a_in{h}")
                for h in range(HPC)
            ]
            a2a_out = [
                dramp.tile([NCORES, D, RBW], bf16, name=f"a2a_out{h}")
                for h in range(HPC)
            ]
            zrow_dram = dramp.tile([PAIRS, N], bf16, name="zrow_dram")
            pvm_dram = dramp.tile([1, DIM], f32, name="pvm_dram")

            with tc.tile_pool(name="constp", bufs=1) as constp:
                wt_sb = constp.tile([P, CT, DIM], bf16, name="wt_sb")
                u_sb = constp.tile([P, RB // P], f32, name="u_sb")
                bout128 = constp.tile([P, DIM], f32, name="bout128")
                gat = constp.tile([P, CT, RBW], bf16, name="gat")

                def prefetch_proj_consts():
                    # emitted after the first pair's loads so they do not
                    # crowd the DMA queues ahead of the critical path
                    for ct in range(CT):
                        nc.sync.dma_start(
                            wt_sb[:, ct, :], wTD[ct * P : (ct + 1) * P, :]
                        )
                    nc.sync.dma_start(u_sb[:], uD[:])
                    nc.sync.dma_start(
                        bout128[:], boutD[0:1, :].to_broadcast((P, DIM))
                    )
                    for vpr in range(PAIRS):
                        vb, vhl = divmod(vpr, HPC)
                        for ic in range(IC):
                            nc.sync.dma_start(
                                a2a_in[vhl][HPC * vb + ic, :, RB : RB + 1],
                                vmD[vpr : vpr + 1, :].rearrange("o d -> d o"),
                            )

                with (
                    tc.tile_pool(name="qkp", bufs=3) as qkp,
                    tc.tile_pool(name="vpool", bufs=3) as vp,
                    tc.tile_pool(name="ptp", bufs=3) as ptp,
                    tc.tile_pool(name="onump", bufs=2) as onp,
                    tc.tile_pool(name="smallp", bufs=2) as smallp,
                    tc.tile_pool(name="finp", bufs=2) as finp,
                    tc.tile_pool(name="psS", bufs=2, space="PSUM") as psS,
                    tc.tile_pool(name="psO", bufs=2, space="PSUM") as psO,
                ):
                    first = True
                    for hl in range(HPC):
                        for b in range(B):
                            pr = b * HPC + hl
                            nk_b = nkb[b]
                            jtk = nk_b // P
                            qt = qkp.tile([P, N], bf16, tag="qt", name=f"qt{pr}")
                            kt = qkp.tile([P, nk], bf16, tag="kt", name=f"kt{pr}")
                            nc.any.memset(qt[D:, :], 0.0)
                            nc.any.memset(kt[D:, :nk_b], 0.0)
                            # split loads: the first S matmul only needs the
                            # leading slices, so it can start sooner (finest
                            # for the very first pair, which gates startup)
                            ksplits = (
                                (0, P, 2 * P, 4 * P, nk_b // 2, nk_b)
                                if first
                                else (0, nk_b // 2, nk_b)
                            )
                            for lo2, hi2 in zip(ksplits[:-1], ksplits[1:]):
                                if lo2 < hi2:
                                    nc.sync.dma_start(
                                        kt[:D, lo2:hi2], kT[pr, :, lo2:hi2]
                                    )
                            qsplits = (0, 512, 1024, N // 2, N) if first else (
                                0, N // 4, N // 2, 3 * N // 4, N
                            )
                            for lo2, hi2 in zip(qsplits[:-1], qsplits[1:]):
                                if lo2 < hi2:
                                    nc.sync.dma_start(
                                        qt[:D, lo2:hi2], qT[pr, :, lo2:hi2]
                                    )
                            vt = vp.tile(
                                [P, jtk, D + 1], bf16, tag="vt", name=f"vt{pr}"
                            )
                            t2 = max(jtk // 2, 1)
                            for lo, hi in ((0, t2), (t2, jtk)):
                                if lo >= hi:
                                    continue
                                nc.sync.dma_start(
                                    vt[:, lo:hi, :],
                                    vv[pr, lo * P : hi * P, :]
                                    .rearrange("(t pp) d -> pp t d", pp=P),
                                )
                            minv_p = smallp.tile(
                                [1, N], f32, tag="minvp", name=f"mi{pr}"
                            )
                            nc.sync.dma_start(minv_p[:], minvD[pr : pr + 1, :])
                            if first:
                                prefetch_proj_consts()
                                first = False

                            for ic in range(IC):
                                i0 = ic * ICW
                                o_ps = psO.tile(
                                    [D + 1, ICW], f32, tag="ops", name=f"o{pr}_{ic}"
                                )
                                for jt in range(jtk):
                                    s_ps = psS.tile(
                                        [P, ICW],
                                        f32,
                                        tag="sps",
                                        name=f"s{pr}_{ic}_{jt}",
                                    )
                                    for n0 in range(0, ICW, 512):
                                        nc.tensor.matmul(
                                            s_ps[:, n0 : n0 + 512],
                                            lhsT=kt[:, jt * P : (jt + 1) * P],
                                            rhs=qt[:, i0 + n0 : i0 + n0 + 512],
                                            start=True,
                                            stop=True,
                                        )
                                    pt = ptp.tile(
                                        [P, ICW],
                                        bf16,
                                        tag="pt",
                                        name=f"p{pr}_{ic}_{jt}",
                                    )
                                    nc.scalar.activation(
                                        pt[:],
                                        s_ps[:],
                                        mybir.ActivationFunctionType.Exp,
                                        scale=SCALE,
                                    )
                                    for n0 in range(0, ICW, 512):
                                        last_pv = nc.tensor.matmul(
                                            o_ps[:, n0 : n0 + 512],
                                            lhsT=vt[:, jt, :],
                                            rhs=pt[:, n0 : n0 + 512],
                                            start=(jt == 0),
                                            stop=(jt == jtk - 1),
                                        )
                                # evacuate PSUM, then the per-chunk z path:
                                # zm = m_i / Z_i (masked queries -> 0), scale
                                # the numerators and ship this chunk at once
                                onum = onp.tile(
                                    [D, ICW], bf16, tag="onum", name=f"on{pr}_{ic}"
                                )
                                nc.vector.tensor_copy(onum[:], o_ps[:D, :])
                                zpair = smallp.tile(
                                    [1, ICW], f32, tag="zpair", name=f"zp{pr}_{ic}"
                                )
                                nc.vector.tensor_copy(zpair[:], o_ps[D : D + 1, :])
                                zq = smallp.tile(
                                    [1, ICW], f32, tag="zq", name=f"zq{pr}_{ic}"
                                )
                                nc.vector.tensor_tensor(
                                    zq[:],
                                    zpair[:],
                                    minv_p[0:1, i0 : i0 + ICW],
                                    mybir.AluOpType.mult,
                                )
                                zr = smallp.tile(
                                    [1, ICW], f32, tag="zr", name=f"zr{pr}_{ic}"
                                )
                                nc.vector.reciprocal_approx_fast(zr[:], zq[:])
                                zrb = smallp.tile(
                                    [1, ICW], bf16, tag="zrb", name=f"zb{pr}_{ic}"
                                )
                                nc.vector.tensor_copy(zrb[:], zr[:])
                                nc.sync.dma_start(
                                    zrow_dram[pr : pr + 1, i0 : i0 + ICW], zrb[:]
                                )
                                zm64 = finp.tile(
                                    [D, ICW], bf16, tag="zm64", name=f"zm{pr}_{ic}"
                                )
                                nc.sync.dma_start(
                                    zm64[:],
                                    zrow_dram[
                                        pr : pr + 1, i0 : i0 + ICW
                                    ].to_broadcast((D, ICW)),
                                )
                                fin = finp.tile(
                                    [D, ICW], bf16, tag="fin", name=f"fi{pr}_{ic}"
                                )
                                last_fin = nc.vector.tensor_tensor(
                                    fin[:], onum[:], zm64[:], mybir.AluOpType.mult
                                )
                                nc.sync.dma_start(
                                    a2a_in[hl][HPC * b + ic, :, 0:RB], fin[:]
                                )

                        # this head-half is complete on every core: exchange
                        # it (the hl=0 round is fully hidden under compute)
                        nc.gpsimd.collective_compute(
                            "AllToAll",
                            mybir.AluOpType.bypass,
                            replica_groups=[list(range(NCORES))],
                            ins=[a2a_in[hl].opt()],
                            outs=[a2a_out[hl].opt()],
                        )

                with (
                    tc.tile_pool(name="outp", bufs=3) as outp,
                    tc.tile_pool(name="smallq", bufs=1) as smallq,
                    tc.tile_pool(name="psP", bufs=2, space="PSUM") as psP,
                    tc.tile_pool(name="psPV", bufs=1, space="PSUM") as psPV,
                    tc.tile_pool(name="psWarm", bufs=1, space="PSUM") as psW,
                ):
                    for h in range(HPC):
                        for ct in range(CT):
                            nc.sync.dma_start(
                                gat[h * D : (h + 1) * D, ct, :],
                                a2a_out[h][ct],
                            )

                    def pin(mm, after, why):
                        tile.add_dep_helper(mm.ins, after.ins, sync=False, reason=why)
                        return mm

                    warm_ps = psW.tile([P, 512], f32, name="warm_ps")
                    last_warm = last_pv
                    NWARM, GRP = 80, 10
                    for wi in range(NWARM):
                        last_warm = pin(
                            nc.tensor.matmul(
                                warm_ps[:],
                                lhsT=wt_sb[:, 0, 0:128],
                                rhs=wt_sb[:, 1, 0:512],
                                start=(wi % GRP == 0),
                                stop=(wi % GRP == GRP - 1),
                            ),
                            last_pv,
                            "warmups bridge the A2A window",
                        )

                    pvm_ps = psPV.tile([1, DIM], f32, name="pvm_ps")
                    for ct in range(CT):
                        for n0 in range(0, DIM, 512):
                            pin(
                                nc.tensor.matmul(
                                    pvm_ps[:, n0 : n0 + 512],
                                    lhsT=gat[:, ct, RB : RB + 1],
                                    rhs=wt_sb[:, ct, n0 : n0 + 512],
                                    start=(ct == 0),
                                    stop=(ct == CT - 1),
                                ),
                                last_warm,
                                "keep warmups ahead in the PE stream",
                            )
                    pvm_row = smallq.tile([1, DIM], f32, name="pvm_row")
                    pin(
                        nc.vector.tensor_copy(pvm_row[:], pvm_ps[:]),
                        last_fin,
                        "projection DVE ops stay behind attention DVE",
                    )
                    nc.sync.dma_start(pvm_dram[:], pvm_row[:])
                    pvm128 = smallq.tile([P, DIM], f32, name="pvm128")
                    nc.sync.dma_start(
                        pvm128[:], pvm_dram[0:1, :].to_broadcast((P, DIM))
                    )

                    for rt in range(RB // P):
                        o_ps = psP.tile([P, DIM], f32, tag="prps", name=f"pr{rt}")
                        for ct in range(CT):
                            for n0 in range(0, DIM, 512):
                                pin(
                                    nc.tensor.matmul(
                                        o_ps[:, n0 : n0 + 512],
                                        lhsT=gat[:, ct, rt * P : (rt + 1) * P],
                                        rhs=wt_sb[:, ct, n0 : n0 + 512],
                                        start=(ct == 0),
                                        stop=(ct == CT - 1),
                                    ),
                                    last_warm,
                                    "keep warmups ahead in the PE stream",
                                )
                        t1 = outp.tile([P, DIM], f32, tag="t1", name=f"t1_{rt}")
                        t1_op = nc.vector.scalar_tensor_tensor(
                            t1[:],
                            in0=pvm128[:],
                            scalar=u_sb[:, rt : rt + 1],
                            in1=bout128[:],
                            op0=mybir.AluOpType.mult,
                            op1=mybir.AluOpType.add,
                        )
                        pin(t1_op, last_fin, "projection DVE stays behind attention")
                        osb = outp.tile([P, DIM], f32, tag="osb", name=f"ob{rt}")
                        nc.vector.tensor_tensor(
                            osb[:], o_ps[:], t1[:], mybir.AluOpType.add
                        )
                        nc.sync.dma_start(outD[rt * P : (rt + 1) * P, :], osb[:])

    nc.compile()
    return nc


def _get_nc(nkb=(N,) * B):
    key = f"nc{nkb}"
    if key not in _CACHE:
        _CACHE[key] = build_graph(nkb)
    return _CACHE[key]


def key_budget(mask):
    """Per-batch compacted key counts (unmasked incl. CLS), padded to 128."""
    counts = 1 + np.asarray(mask).astype(bool).sum(axis=1)
    return tuple(
        min(max(int(-(-int(c) // P) * P), P), N) for c in counts
    )


def make_in_maps(q, k, v, mask, W_out, b_out, nkb=(N,) * B):
    nk = max(nkb)
    q16 = q.astype(npbf)
    k16 = k.astype(npbf)
    v16 = v.astype(npbf)
    m_full = np.concatenate(
        [np.ones((B, 1), dtype=bool), np.asarray(mask).astype(bool)], axis=1
    )  # [B, N]
    # key compaction: keep only unmasked keys (masked ones contribute
    # exp(-inf)=0 exactly); pad each batch to nk with bias-killed slots
    kC = np.zeros((B, H, nk, D), dtype=npbf)
    vC = np.zeros((B, H, nk, D + 1), dtype=npbf)
    for b in range(B):
        idx = np.flatnonzero(m_full[b])
        kC[b, :, : len(idx)] = k16[b][:, idx, :]
        vC[b, :, : len(idx), :D] = v16[b][:, idx, :]
        vC[b, :, : len(idx), D] = 1.0
    minv = np.where(m_full, 1.0, MBIG).astype(np.float32)[
        np.repeat(np.arange(B), HPC)
    ]  # [PAIRS, N]
    wT16 = np.ascontiguousarray(np.asarray(W_out).T).astype(npbf)
    bout = np.asarray(b_out).astype(np.float32).reshape(1, DIM)

    in_maps = []
    for c in range(NCORES):
        heads = slice(HPC * c, HPC * (c + 1))
        qTc = np.ascontiguousarray(
            q16[:, heads].transpose(0, 1, 3, 2).reshape(PAIRS, D, N)
        )
        kTc = np.ascontiguousarray(
            kC[:, heads].transpose(0, 1, 3, 2).reshape(PAIRS, D, nk)
        )
        vc = np.ascontiguousarray(vC[:, heads].reshape(PAIRS, nk, D + 1))
        vmc = (
            v16[:, heads].astype(np.float32).sum(axis=2).reshape(PAIRS, D)
        ).astype(npbf)
        bc = c // 2
        i0c = (c % 2) * RB
        u_core = np.ascontiguousarray(
            ((1.0 - m_full[bc, i0c : i0c + RB].astype(np.float32)) / N)
            .reshape(RB // P, P)
            .T
        )
        in_maps.append(
            {
                "qT": qTc,
                "kT": kTc,
                "v": vc,
                "minv": minv,
                "uproj": u_core,
                "wT": wT16,
                "vmean": vmc,
                "bout": bout,
            }
        )
    return in_maps


def run(q, k, v, mask, W_out, b_out, trace=False, **spmd_kwargs):
    nkb = key_budget(mask)
    nc = _get_nc(nkb)
    in_maps = make_in_maps(q, k, v, mask, W_out, b_out, nkb=nkb)
    res = run_bass_kernel_spmd(
        nc, in_maps, core_ids=list(range(NCORES)), trace=trace, **spmd_kwargs
    )
    outs = [np.asarray(res.results[c]["out"]) for c in range(NCORES)]
    full = np.concatenate(outs, axis=0).reshape(B, N, DIM).astype(np.float32)
    return full, res


def kernel(q, k, v, mask, W_out, b_out):
    out, _ = run(q, k, v, mask, W_out, b_out, trace=False)
    return out



# revision 4
# speedup vs baseline: 1.0580x; 1.0580x over previous
"""Distributed Trainium2 (8 NeuronCores) kernel for masked multi-head attention
+ output projection (nn_Attention_60790967107825).

v2: query compaction on top of the v1 key compaction.

The reference masks queries and keys with the same per-batch mask; masked
query rows collapse to uniform attention over ALL keys (a rank-1-per-batch
term).  v1 compacted keys only; v2 also compacts queries, so the whole
attention pipeline (scores, exp, PV, normalization, A2A payload, output
projection) runs on the ~50% surviving rows:

  - Each core owns 2 of 16 heads x 4 batches = 8 (b, h) pairs.  Per batch,
    the host keeps only unmasked rows (CLS + mask), in original order, so
    compacted queries split cleanly at c0_b into dest-core halves.
  - Scores are computed transposed (S^T = K Q^T, 64-deep contraction, no
    zero-padding memsets) in query chunks of <=1024 columns; the few
    overflow queries beyond 1024 pack all their key tiles into one PSUM
    tile so they cost a single exp instruction.
  - Z comes free from a ones-column appended to V; normalization is
    1/Z broadcast (DVE fast reciprocal -> DRAM round-trip partition
    broadcast in f32) fused into the PSUM evacuation multiply.
  - Two AllToAlls (~0.6MB/rank), one per head-half; #0 hides under the
    second half of attention, #1 is bridged by PE warmups pinned to the
    last fin DMA so they burn clock exactly during the exchange.
  - Each core projects its received compacted rows (<=640) against the
    full 1024x1024 W; masked rows are reconstructed on the host from a
    per-batch V-sum row carried through the collective and projected on
    device (pvm output).  Host scatters compacted rows back to their
    original positions and broadcast-fills masked rows.
"""

import os
import sys

import numpy as np

for _p in ("/opt/trn_rl_repo", "/root/.axon_site/_ro/trn_rl_repo"):
    if os.path.isdir(_p) and _p not in sys.path:
        sys.path.insert(0, _p)

import ml_dtypes  # noqa: E402
import concourse.bass as bass  # noqa: E402,F401
import concourse.mybir as mybir  # noqa: E402
import concourse.tile as tile  # noqa: E402
from concourse import bacc  # noqa: E402
from concourse.bass_utils import run_bass_kernel_spmd  # noqa: E402

B, H, N, D = 4, 16, 2048, 64
DIM = H * D
P = 128
NCORES = 8
HPC = H // NCORES          # heads per core
PAIRS = B * HPC            # (b, h_local) pairs per core
SCALE = float(D) ** -0.5
CT = DIM // P              # 8 contraction tiles in the projection
CHUNK = 1024               # query-chunk width (2 PSUM banks of f32)

bf16 = mybir.dt.bfloat16
f32 = mybir.dt.float32
npbf = ml_dtypes.bfloat16

_CACHE = {}


def _plan(c0s, c1s):
    """Derived sizes shared by graph builder and host prep."""
    Ms = tuple(a + b for a, b in zip(c0s, c1s))
    Mks = tuple(-(-m // P) * P for m in Ms)
    NQ = max(Ms)
    NK = max(Mks)
    CMAX = max(max(c0s), max(c1s))
    RP = -(-CMAX // P) * P          # projection rows per core (padded)
    VMCOL = -(-CMAX // 8) * 8       # v-sum column offset in a2a slots
    AW = VMCOL + 8                  # a2a slot width
    return Ms, Mks, NQ, NK, CMAX, RP, VMCOL, AW


def build_graph(c0s, c1s):
    Ms, Mks, NQ, NK, CMAX, RP, VMCOL, AW = _plan(c0s, c1s)
    JTK_MAX = max(Mks) // P
    nc = bacc.Bacc("TRN2", num_devices=NCORES)

    qT = nc.dram_tensor("qT", [PAIRS, D, NQ], bf16, kind="ExternalInput")
    kT = nc.dram_tensor("kT", [PAIRS, D, NK], bf16, kind="ExternalInput")
    vv = nc.dram_tensor("v", [PAIRS, NK, D + 1], bf16, kind="ExternalInput")
    wTD = nc.dram_tensor("wT", [DIM, DIM], bf16, kind="ExternalInput")
    boutD = nc.dram_tensor("bout", [1, DIM], f32, kind="ExternalInput")
    vmD = nc.dram_tensor("vmean", [PAIRS, D], bf16, kind="ExternalInput")
    outD = nc.dram_tensor("out", [RP, DIM], f32, kind="ExternalOutput")
    pvmD = nc.dram_tensor("pvm", [1, DIM], f32, kind="ExternalOutput")

    with tile.TileContext(nc, num_cores=NCORES) as tc:
        with tc.tile_pool(name="dram", bufs=1, space="DRAM") as dramp:
            a2a_in = [
                dramp.tile([NCORES, D, AW], bf16, name=f"a2a_in{h}")
                for h in range(HPC)
            ]
            a2a_out = [
                dramp.tile([NCORES, D, AW], bf16, name=f"a2a_out{h}")
                for h in range(HPC)
            ]
            zrow_dram = dramp.tile([PAIRS, NQ], f32, name="zrow_dram")

            with tc.tile_pool(name="constp", bufs=1) as constp:
                wt_sb = constp.tile([P, CT, DIM], bf16, name="wt_sb")
                bout128 = constp.tile([P, DIM], f32, name="bout128")
                gat = constp.tile([P, CT, RP + 8], bf16, name="gat")

                def prefetch_proj_consts():
                    # emitted after the first pair's loads so they do not
                    # crowd the DMA queues ahead of the critical path
                    for ct in range(CT):
                        nc.sync.dma_start(
                            wt_sb[:, ct, :], wTD[ct * P : (ct + 1) * P, :]
                        )
                    nc.sync.dma_start(
                        bout128[:], boutD[0:1, :].to_broadcast((P, DIM))
                    )
                    for vpr in range(PAIRS):
                        vb, vhl = divmod(vpr, HPC)
                        for half in range(2):
                            nc.sync.dma_start(
                                a2a_in[vhl][2 * vb + half, :, VMCOL : VMCOL + 1],
                                vmD[vpr : vpr + 1, :].rearrange("o d -> d o"),
                            )

                with (
                    tc.tile_pool(name="qkp", bufs=3) as qkp,
                    tc.tile_pool(name="vpool", bufs=3) as vp,
                    tc.tile_pool(name="ptp", bufs=3) as ptp,
                    tc.tile_pool(name="smallp", bufs=2) as smallp,
                    tc.tile_pool(name="finp", bufs=2) as finp,
                    tc.tile_pool(name="psS", bufs=2, space="PSUM") as psS,
                    tc.tile_pool(name="psO", bufs=2, space="PSUM") as psO,
                ):
                    first = True
                    last_fin_dma = None
                    for hl in range(HPC):
                        for b in range(B):
                            pr = b * HPC + hl
                            M, Mk, c0 = Ms[b], Mks[b], c0s[b]
                            jtk = Mk // P
                            W1 = min(M, CHUNK)
                            W2 = M - W1
                            qt = qkp.tile([D, NQ], bf16, tag="qt", name=f"qt{pr}")
                            kt = qkp.tile([D, NK], bf16, tag="kt", name=f"kt{pr}")
                            # split loads: the first S matmul only needs the
                            # leading slices, so it can start sooner
                            ksplits = (
                                (0, P, 2 * P, 4 * P, Mk // 2, Mk)
                                if first
                                else (0, Mk // 2, Mk)
                            )
                            for lo2, hi2 in zip(ksplits[:-1], ksplits[1:]):
                                if lo2 < hi2:
                                    nc.sync.dma_start(
                                        kt[:, lo2:hi2], kT[pr, :, lo2:hi2]
                                    )
                            qsplits = (
                                (0, 256, 512, W1, M) if first else (0, M // 2, M)
                            )
                            for lo2, hi2 in zip(qsplits[:-1], qsplits[1:]):
                                if lo2 < hi2:
                                    nc.sync.dma_start(
                                        qt[:, lo2:hi2], qT[pr, :, lo2:hi2]
                                    )
                            vt = vp.tile(
                                [P, JTK_MAX, D + 1], bf16, tag="vt", name=f"vt{pr}"
                            )
                            t2 = max(jtk // 2, 1)
                            for lo, hi in ((0, t2), (t2, jtk)):
                                if lo >= hi:
                                    continue
                                nc.sync.dma_start(
                                    vt[:, lo:hi, :],
                                    vv[pr, lo * P : hi * P, :]
                                    .rearrange("(t pp) d -> pp t d", pp=P),
                                )
                            if first:
                                prefetch_proj_consts()
                                first = False

                            # ---- chunk 1: query columns [0, W1) ----
                            o1 = psO.tile(
                                [D + 1, CHUNK], f32, tag="ops", name=f"o1_{pr}"
                            )
                            for jt in range(jtk):
                                s_ps = psS.tile(
                                    [P, CHUNK], f32, tag="sps", name=f"s{pr}_{jt}"
                                )
                                for n0 in range(0, W1, 512):
                                    w = min(512, W1 - n0)
                                    nc.tensor.matmul(
                                        s_ps[:, n0 : n0 + w],
                                        lhsT=kt[:, jt * P : (jt + 1) * P],
                                        rhs=qt[:, n0 : n0 + w],
                                        start=True,
                                        stop=True,
                                    )
                                pt = ptp.tile(
                                    [P, CHUNK], bf16, tag="pt", name=f"p{pr}_{jt}"
                                )
                                nc.scalar.activation(
                                    pt[:, 0:W1],
                                    s_ps[:, 0:W1],
                                    mybir.ActivationFunctionType.Exp,
                                    scale=SCALE,
                                )
                                for n0 in range(0, W1, 512):
                                    w = min(512, W1 - n0)
                                    last_pv = nc.tensor.matmul(
                                        o1[:, n0 : n0 + w],
                                        lhsT=vt[:, jt, :],
                                        rhs=pt[:, n0 : n0 + w],
                                        start=(jt == 0),
                                        stop=(jt == jtk - 1),
                                    )
                            # z + evacuation for chunk 1 (overlaps chunk 2)
                            zp1 = smallp.tile(
                                [1, CHUNK], f32, tag="zp", name=f"zp1_{pr}"
                            )
                            nc.vector.tensor_copy(zp1[:, 0:W1], o1[D : D + 1, 0:W1])
                            zr1 = smallp.tile(
                                [1, CHUNK], f32, tag="zr", name=f"zr1_{pr}"
                            )
                            nc.vector.reciprocal_approx_fast(
                                zr1[:, 0:W1], zp1[:, 0:W1]
                            )
                            nc.sync.dma_start(
                                zrow_dram[pr : pr + 1, 0:W1], zr1[:, 0:W1]
                            )
                            zm1 = finp.tile(
                                [D, CHUNK], f32, tag="zm", name=f"zm1_{pr}"
                            )
                            nc.sync.dma_start(
                                zm1[:, 0:W1],
                                zrow_dram[pr : pr + 1, 0:W1].to_broadcast((D, W1)),
                            )
                            fin1 = finp.tile(
                                [D, CHUNK], bf16, tag="fin", name=f"fi1_{pr}"
                            )
                            last_fin = nc.vector.tensor_tensor(
                                fin1[:, 0:W1],
                                o1[0:D, 0:W1],
                                zm1[:, 0:W1],
                                mybir.AluOpType.mult,
                            )
                            nc.sync.dma_start(
                                a2a_in[hl][2 * b, :, 0:c0], fin1[:, 0:c0]
                            )
                            last_fin_dma = nc.sync.dma_start(
                                a2a_in[hl][2 * b + 1, :, 0 : W1 - c0],
                                fin1[:, c0:W1],
                            )

                            # ---- chunk 2: overflow queries [W1, M) ----
                            if W2 > 0:
                                o2 = psO.tile(
                                    [D + 1, CHUNK], f32, tag="ops", name=f"o2_{pr}"
                                )
                                s2 = psS.tile(
                                    [P, CHUNK], f32, tag="sps", name=f"s2_{pr}"
                                )
                                for jt in range(jtk):
                                    nc.tensor.matmul(
                                        s2[:, jt * W2 : (jt + 1) * W2],
                                        lhsT=kt[:, jt * P : (jt + 1) * P],
                                        rhs=qt[:, W1:M],
                                        start=True,
                                        stop=True,
                                    )
                                pt2 = ptp.tile(
                                    [P, 256], bf16, tag="pt2", name=f"p2_{pr}"
                                )
                                nc.scalar.activation(
                                    pt2[:, 0 : jtk * W2],
                                    s2[:, 0 : jtk * W2],
                                    mybir.ActivationFunctionType.Exp,
                                    scale=SCALE,
                                )
                                for jt in range(jtk):
                                    last_pv = nc.tensor.matmul(
                                        o2[:, 0:W2],
                                        lhsT=vt[:, jt, :],
                                        rhs=pt2[:, jt * W2 : (jt + 1) * W2],
                                        start=(jt == 0),
                                        stop=(jt == jtk - 1),
                                    )
                                zp2 = smallp.tile(
                                    [1, CHUNK], f32, tag="zp", name=f"zp2_{pr}"
                                )
                                nc.vector.tensor_copy(
                                    zp2[:, 0:W2], o2[D : D + 1, 0:W2]
                                )
                                zr2 = smallp.tile(
                                    [1, CHUNK], f32, tag="zr", name=f"zr2_{pr}"
                                )
                                nc.vector.reciprocal_approx_fast(
                                    zr2[:, 0:W2], zp2[:, 0:W2]
                                )
                                nc.sync.dma_start(
                                    zrow_dram[pr : pr + 1, W1:M], zr2[:, 0:W2]
                                )
                                zm2 = finp.tile(
                                    [D, 256], f32, tag="zm2", name=f"zm2_{pr}"
                                )
                                nc.sync.dma_start(
                                    zm2[:, 0:W2],
                                    zrow_dram[pr : pr + 1, W1:M].to_broadcast(
                                        (D, W2)
                                    ),
                                )
                                fin2 = finp.tile(
                                    [D, 256], bf16, tag="fin2", name=f"fi2_{pr}"
                                )
                                last_fin = nc.vector.tensor_tensor(
                                    fin2[:, 0:W2],
                                    o2[0:D, 0:W2],
                                    zm2[:, 0:W2],
                                    mybir.AluOpType.mult,
                                )
                                last_fin_dma = nc.sync.dma_start(
                                    a2a_in[hl][
                                        2 * b + 1, :, W1 - c0 : W1 - c0 + W2
                                    ],
                                    fin2[:, 0:W2],
                                )

                        # this head-half is complete on every core: exchange
                        # it (the hl=0 round is fully hidden under compute)
                        nc.gpsimd.collective_compute(
                            "AllToAll",
                            mybir.AluOpType.bypass,
                            replica_groups=[list(range(NCORES))],
                            ins=[a2a_in[hl].opt()],
                            outs=[a2a_out[hl].opt()],
                        )

                with (
                    tc.tile_pool(name="outp", bufs=3) as outp,
                    tc.tile_pool(name="smallq", bufs=1) as smallq,
                    tc.tile_pool(name="psP", bufs=2, space="PSUM") as psP,
                    tc.tile_pool(name="psPV", bufs=1, space="PSUM") as psPV,
                    tc.tile_pool(name="psWarm", bufs=1, space="PSUM") as psW,
                ):
                    for h in range(HPC):
                        for ct in range(CT):
                            nc.sync.dma_start(
                                gat[h * D : (h + 1) * D, ct, 0:AW],
                                a2a_out[h][ct],
                            )

                    def pin(mm, after, why):
                        tile.add_dep_helper(mm.ins, after.ins, sync=False, reason=why)
                        return mm

                    # warmups pinned to the LAST a2a-in DMA: they start when
                    # the exchange starts and keep the PE clock at full HAM
                    # through the collective window
                    warm_ps = psW.tile([P, 512], f32, name="warm_ps")
                    last_warm = last_pv
                    NWARM, GRP = 56, 8
                    for wi in range(NWARM):
                        last_warm = pin(
                            nc.tensor.matmul(
                                warm_ps[:],
                                lhsT=wt_sb[:, 0, 0:128],
                                rhs=wt_sb[:, 1, 0:512],
                                start=(wi % GRP == 0),
                                stop=(wi % GRP == GRP - 1),
                            ),
                            last_fin_dma,
                            "warmups bridge the A2A window",
                        )

                    pvm_ps = psPV.tile([1, DIM], f32, name="pvm_ps")
                    for ct in range(CT):
                        for n0 in range(0, DIM, 512):
                            pin(
                                nc.tensor.matmul(
                                    pvm_ps[:, n0 : n0 + 512],
                                    lhsT=gat[:, ct, VMCOL : VMCOL + 1],
                                    rhs=wt_sb[:, ct, n0 : n0 + 512],
                                    start=(ct == 0),
                                    stop=(ct == CT - 1),
                                ),
                                last_warm,
                                "keep warmups ahead in the PE stream",
                            )
                    pvm_row = smallq.tile([1, DIM], f32, name="pvm_row")
                    pin(
                        nc.vector.tensor_copy(pvm_row[:], pvm_ps[:]),
                        last_fin,
                        "projection DVE ops stay behind attention DVE",
                    )
                    nc.sync.dma_start(pvmD[:], pvm_row[:])

                    for rt in range(RP // P):
                        o_ps = psP.tile([P, DIM], f32, tag="prps", name=f"pr{rt}")
                        for ct in range(CT):
                            for n0 in range(0, DIM, 512):
                                pin(
                                    nc.tensor.matmul(
                                        o_ps[:, n0 : n0 + 512],
                                        lhsT=gat[:, ct, rt * P : (rt + 1) * P],
                                        rhs=wt_sb[:, ct, n0 : n0 + 512],
                                        start=(ct == 0),
                                        stop=(ct == CT - 1),
                                    ),
                                    last_warm,
                                    "keep warmups ahead in the PE stream",
                                )
                        osb = outp.tile([P, DIM], f32, tag="osb", name=f"ob{rt}")
                        pin(
                            nc.vector.tensor_tensor(
                                osb[:], o_ps[:], bout128[:], mybir.AluOpType.add
                            ),
                            last_fin,
                            "projection DVE stays behind attention",
                        )
                        nc.sync.dma_start(outD[rt * P : (rt + 1) * P, :], osb[:])

    nc.compile()
    return nc


def _get_nc(c0s, c1s):
    key = (c0s, c1s)
    if key not in _CACHE:
        _CACHE[key] = build_graph(c0s, c1s)
    return _CACHE[key]


def mask_plan(mask):
    """Per-batch compacted-row indices and half counts."""
    m_full = np.concatenate(
        [np.ones((B, 1), dtype=bool), np.asarray(mask).astype(bool)], axis=1
    )  # [B, N]
    idxs = [np.flatnonzero(m_full[b]) for b in range(B)]
    c0s = tuple(int((i < N // 2).sum()) for i in idxs)
    c1s = tuple(len(i) - c for i, c in zip(idxs, c0s))
    return m_full, idxs, c0s, c1s


def make_in_maps(q, k, v, mask, W_out, b_out, idxs, c0s, c1s):
    Ms, Mks, NQ, NK, CMAX, RP, VMCOL, AW = _plan(c0s, c1s)
    q16 = np.asarray(q).astype(npbf)
    k16 = np.asarray(k).astype(npbf)
    v16 = np.asarray(v).astype(npbf)

    # compacted per batch: queries exact-width, keys padded to Mk with
    # zeros (zero keys score exp(0)=1 but carry 0 in the V ones-column,
    # so they add nothing to numerator or Z)
    qC = np.zeros((B, H, NQ, D), dtype=npbf)
    kC = np.zeros((B, H, NK, D), dtype=npbf)
    vC = np.zeros((B, H, NK, D + 1), dtype=npbf)
    for b in range(B):
        idx = idxs[b]
        qC[b, :, : len(idx)] = q16[b][:, idx, :]
        kC[b, :, : len(idx)] = k16[b][:, idx, :]
        vC[b, :, : len(idx), :D] = v16[b][:, idx, :]
        vC[b, :, : len(idx), D] = 1.0
    wT16 = np.ascontiguousarray(np.asarray(W_out).T).astype(npbf)
    bout = np.asarray(b_out).astype(np.float32).reshape(1, DIM)

    in_maps = []
    for c in range(NCORES):
        heads = slice(HPC * c, HPC * (c + 1))
        qTc = np.ascontiguousarray(
            qC[:, heads].transpose(0, 1, 3, 2).reshape(PAIRS, D, NQ)
        )
        kTc = np.ascontiguousarray(
            kC[:, heads].transpose(0, 1, 3, 2).reshape(PAIRS, D, NK)
        )
        vc = np.ascontiguousarray(vC[:, heads].reshape(PAIRS, NK, D + 1))
        vmc = (
            v16[:, heads].astype(np.float32).sum(axis=2).reshape(PAIRS, D)
        ).astype(npbf)
        in_maps.append(
            {
                "qT": qTc,
                "kT": kTc,
                "v": vc,
                "wT": wT16,
                "vmean": vmc,
                "bout": bout,
            }
        )
    return in_maps


def run(q, k, v, mask, W_out, b_out, trace=False, **spmd_kwargs):
    m_full, idxs, c0s, c1s = mask_plan(mask)
    nc = _get_nc(c0s, c1s)
    in_maps = make_in_maps(q, k, v, mask, W_out, b_out, idxs, c0s, c1s)
    res = run_bass_kernel_spmd(
        nc, in_maps, core_ids=list(range(NCORES)), trace=trace, **spmd_kwargs
    )
    bout = np.asarray(b_out).astype(np.float32).reshape(DIM)
    full = np.empty((B, N, DIM), dtype=np.float32)
    for b in range(B):
        r0 = np.asarray(res.results[2 * b]["out"])[: c0s[b]]
        r1 = np.asarray(res.results[2 * b + 1]["out"])[: c1s[b]]
        full[b, idxs[b]] = np.concatenate([r0, r1], axis=0)
        pvm = np.asarray(res.results[2 * b]["pvm"])[0]
        full[b, ~m_full[b]] = pvm * (1.0 / N) + bout
    return full, res


def kernel(q, k, v, mask, W_out, b_out):
    out, _ = run(q, k, v, mask, W_out, b_out, trace=False)
    return out
